# revision 1
# baseline (speedup 1.0000x reference)
"""ChebConv (K=3) Trainium2 Bass kernel — 8-core data-parallel.

Reference computation (bug-faithful torch .view semantics):
    A[b,k]   = T_k(L_b)          (T0=I, T1=L, T2=2 L@L - I),  A: [B,3,9,9]
    R        = A.reshape(3,B,9,9)               (raw reshape == scramble)
    out[b']  = sum_k (R[k,b'] @ X[b']) @ W[k]  + bias
with B=8192, N=9, C_in=C_out=1024.

Strategy per core (1024 samples, padded to 1036 = 74 groups x 14 samples):
  Host:  build per-group block-diagonal "cheb" moving operands
         BD[g][9l+j, PW*k+9l+n] = R[k, b_l]^T[j, n]   (bf16, zeros included),
         cast X and W to bf16, restack W by (k, m-chunk).
  T-apply (PE):  psum[m,(k,l,n)] = matmul(lhsT=X_grp[126,128mc], rhs=BD[126,3PW])
                 -> P^T directly (partition = channel), cast to bf16 in SBUF.
  Big matmul (PE): out[PW,1024] accumulates 24 chunks:
                 matmul(lhsT=P^T chunk [128,PW] stationary, rhs=W chunk [128,512]).
  Evacuate PSUM + bias add, contiguous DMA out. fp32 output.

PW=128 (pad (l,n) 126->128 with zeros) so the big-matmul stationary loads have
exactly 128 columns, enabling the PE's fast-weight-load path.
"""

import numpy as np
import ml_dtypes

import concourse.bass as bass
import concourse.mybir as mybir
import concourse.tile as tile
from concourse import bacc
from concourse.bass_utils import run_bass_kernel_spmd

BF16 = mybir.dt.bfloat16
F32 = mybir.dt.float32
NP_BF16 = ml_dtypes.bfloat16

B, N, C = 8192, 9, 1024
NCORES = 8
BC = B // NCORES          # 1024 samples per core
GS = 14                   # samples per group (14*9 = 126 partitions)
GROUPS = 74               # ceil(1024/14) -> pad to 1036
BCP = GROUPS * GS         # 1036
ROWS = GS * N             # 126
KCH = 24                  # contraction chunks: 3 k * 8 m-chunks


def build_module(groups=GROUPS, repeats=1, pad128=True, out_evac="vector",
                 no_t=False, psum_split=(4, 2), pt_engine="vector",
                 t_slots=(6, 8, 10, 12, 14, 16, 18, 20), out_dma="sync"):
    PW = 128 if pad128 else ROWS      # per-k chunk width in PT
    CW = 3 * PW                       # T-apply psum width (k planes)

    nc = bacc.Bacc("TRN2", target_bir_lowering=False, debug=False,
                   num_devices=NCORES)

    x_d = nc.dram_tensor("x", [groups * ROWS, C], BF16, kind="ExternalInput")
    bd_d = nc.dram_tensor("bd", [groups, ROWS, CW], BF16, kind="ExternalInput")
    w_d = nc.dram_tensor("w", [128, KCH * C], BF16, kind="ExternalInput")
    bias_d = nc.dram_tensor("bias", [ROWS, C], F32, kind="ExternalInput")
    out_d = nc.dram_tensor("out", [groups * ROWS, C], F32, kind="ExternalOutput")

    with tile.TileContext(nc) as tc:
        with (
            tc.tile_pool(name="const", bufs=1) as cpool,
            tc.tile_pool(name="x", bufs=3) as xpool,
            tc.tile_pool(name="bdp", bufs=3) as bdpool,
            tc.tile_pool(name="pt", bufs=2) as ptpool,
            tc.tile_pool(name="osb", bufs=3) as opool,
            tc.tile_pool(name="ptps", bufs=psum_split[0],
                         space=bass.MemorySpace.PSUM) as ptpsum,
            tc.tile_pool(name="ops", bufs=psum_split[1],
                         space=bass.MemorySpace.PSUM) as opsum,
        ):
            w_sb = cpool.tile([128, KCH * C], BF16, tag="w")
            nc.sync.dma_start(w_sb[:], w_d[:])
            bias_sb = cpool.tile([ROWS, C], F32, tag="bias")
            nc.sync.dma_start(bias_sb[:], bias_d[:])
            if no_t:
                pt_const = cpool.tile([128, 8 * CW], BF16, tag="ptc")
                nc.gpsimd.memset(pt_const[:], 0.5)

            def emit_dma_stage(g):
                """Issue group g's X/BD loads (prefetched ~2 groups ahead)."""
                if no_t:
                    return None
                x_sb = xpool.tile([ROWS, C], BF16, tag="x")
                nc.sync.dma_start(x_sb[:], x_d[g * ROWS:(g + 1) * ROWS, :])
                bd_sb = bdpool.tile([ROWS, CW], BF16, tag="bd")
                nc.sync.dma_start(bd_sb[:], bd_d[g])
                return x_sb, bd_sb

            def emit_t_stage(tiles):
                """Return (pt_sb, [thunks]); each thunk emits one T-apply
                matmul + its PSUM->SBUF bf16 evacuation."""
                if no_t:
                    return pt_const, [(lambda: None) for _ in range(8)]
                x_sb, bd_sb = tiles
                pt_sb = ptpool.tile([128, 8 * CW], BF16, tag="pt")

                def mk(mc):
                    def thunk():
                        ps = ptpsum.tile([128, CW], F32, tag="ptps")
                        nc.tensor.matmul(
                            ps[:], x_sb[:, mc * 128:(mc + 1) * 128], bd_sb[:],
                            start=True, stop=True)
                        if pt_engine == "scalar" or (
                                pt_engine == "split" and mc % 2 == 0):
                            nc.scalar.copy(
                                pt_sb[:, mc * CW:(mc + 1) * CW], ps[:])
                        else:
                            nc.vector.tensor_copy(
                                pt_sb[:, mc * CW:(mc + 1) * CW], ps[:])
                    return thunk

                return pt_sb, [mk(mc) for mc in range(8)]

            # T-apply matmuls for group g+1 are interleaved into group g's
            # big-matmul stream, in its back half (q >= 8) so the prefetched
            # DMAs have landed and the in-order PE queue never stalls.
            T_SLOTS = t_slots

            def emit_pass():
                dmas = [emit_dma_stage(0), emit_dma_stage(1)]
                pt_cur, thunks = emit_t_stage(dmas[0])
                for t in thunks:
                    t()
                for g in range(groups):
                    nxt = emit_t_stage(dmas[g + 1]) if g + 1 < groups else None
                    out_ps = opsum.tile([PW, C], F32, tag="ops")
                    ti = 0
                    for q in range(KCH):
                        k, mc = divmod(q, 8)
                        lhsT = pt_cur[:, mc * CW + PW * k: mc * CW + PW * (k + 1)]
                        for h in range(2):
                            nc.tensor.matmul(
                                out_ps[:, h * 512:(h + 1) * 512],
                                lhsT,
                                w_sb[:, q * C + h * 512: q * C + (h + 1) * 512],
                                start=(q == 0), stop=(q == KCH - 1))
                        if q == 2 and g + 2 < groups:
                            dmas.append(emit_dma_stage(g + 2))
                        if nxt is not None and q in T_SLOTS:
                            nxt[1][ti]()
                            ti += 1
                    out_sb = opool.tile([ROWS, C], F32, tag="osb")
                    if out_evac == "vector":
                        nc.vector.tensor_add(out_sb[:], out_ps[:ROWS, :],
                                             bias_sb[:])
                    elif out_evac == "act_bias":
                        # ScalarE frees the PSUM bank fast; DVE adds bias
                        # SBUF-side, off the PE<->PSUM dependency chain.
                        nc.scalar.copy(out_sb[:], out_ps[:ROWS, :])
                        nc.vector.tensor_add(out_sb[:], out_sb[:], bias_sb[:])
                    else:
                        nc.scalar.copy(out_sb[:], out_ps[:ROWS, :])
                    out_eng = nc.gpsimd if out_dma == "gpsimd" else nc.sync
                    out_eng.dma_start(out_d[g * ROWS:(g + 1) * ROWS, :], out_sb[:])
                    if nxt is not None:
                        pt_cur = nxt[0]

            if repeats == 1:
                emit_pass()
            else:
                with tc.For_i(0, repeats, 1):
                    emit_pass()

    nc.compile()
    return nc


def prepare_inputs(inputs, mul_data, weight, bias, groups=GROUPS, pad128=True):
    """Host-side layout prep. Returns in_maps (one dict per core)."""
    PW = 128 if pad128 else ROWS
    CW = 3 * PW
    X = np.asarray(inputs, np.float32)
    L = np.asarray(mul_data, np.float32)
    W = np.asarray(weight, np.float32).reshape(3, C, C)
    bias = np.asarray(bias, np.float32).reshape(C)

    bcp = groups * GS

    # Chebyshev blocks + the torch .view scramble, transposed per-block.
    I9 = np.eye(N, dtype=np.float32)
    T2 = 2.0 * np.matmul(L, L) - I9
    A = np.stack([np.broadcast_to(I9, L.shape), L, T2], axis=1)  # [B,3,9,9]
    R = A.reshape(3, B, N, N)
    RT = R.transpose(0, 1, 3, 2)  # RT[k,b',j,n] = R[k,b'][n,j]

    # Per-core slices, padded along samples.
    Xp = np.zeros((NCORES, bcp, N, C), np.float32)
    Xp[:, :BC] = X.reshape(NCORES, BC, N, C)
    x_dev = np.ascontiguousarray(Xp.reshape(NCORES, bcp * N, C)).astype(NP_BF16)

    RTp = np.zeros((3, NCORES, bcp, N, N), np.float32)
    RTp[:, :, :BC] = RT.reshape(3, NCORES, BC, N, N)
    RTg = RTp.transpose(1, 0, 2, 3, 4).reshape(NCORES, 3, groups, GS, N, N)

    BD = np.zeros((NCORES, groups, ROWS, CW), np.float32)
    for k in range(3):
        for l in range(GS):
            BD[:, :, N * l:N * (l + 1),
               PW * k + N * l: PW * k + N * (l + 1)] = RTg[:, k, :, l]
    bd_dev = BD.astype(NP_BF16)

    w_dev = np.ascontiguousarray(
        W.reshape(3, 8, 128, C).transpose(2, 0, 1, 3).reshape(128, KCH * C)
    ).astype(NP_BF16)

    bias_dev = np.ascontiguousarray(
        np.broadcast_to(bias[None, :], (ROWS, C))).astype(np.float32)

    return [
        {"x": x_dev[c], "bd": bd_dev[c], "w": w_dev, "bias": bias_dev}
        for c in range(NCORES)
    ]


_NC_CACHE = {}


def get_module(groups=GROUPS, repeats=1, **kw):
    key = (groups, repeats, tuple(sorted(kw.items())))
    if key not in _NC_CACHE:
        _NC_CACHE[key] = build_module(groups, repeats, **kw)
    return _NC_CACHE[key]


def kernel(inputs, graph, mul_data, weight, bias):
    nc = get_module()
    in_maps = prepare_inputs(inputs, mul_data, weight, bias)
    res = run_bass_kernel_spmd(nc, in_maps, core_ids=list(range(NCORES)))
    outs = [
        res.results[c]["out"][:BC * N].reshape(BC, N, C)
        for c in range(NCORES)
    ]
    return np.concatenate(outs, axis=0)

